# revision 10
# baseline (speedup 1.0000x reference)
"""Trainium2 Bass kernel for nn_CorrectorEGNN (B=128 graphs, N=64 nodes, H=128, L=4).

Strategy: data-parallel over graphs (16 graphs/core x 8 cores). Per graph the
fully-connected edge set is dense 64x64 ordered pairs (i-major, e = i*64+j,
src=i, dst=j). Edge tensors live channel-transposed: [128 chan (partitions),
4096 edges (free)], so the edge MLP is weight-stationary matmuls.

v1 (bf16) changes vs the fp32r baseline:
  - all big matmuls in bf16 (1 cyc/col vs fp32r's 2-pass + HAM-cold penalty)
  - |p|^2 folded into the edge-MLP first-layer lhsT (A' + gd*wrow), Gram term
    as a K=1 rank-1 matmul against the flattened Gram row
  - SBUF->SBUF DMA reshapes (Gram -> g_row, cw row -> CWM); no DRAM bounces
  - no diagonal zeroing of CWM (the i==j term cancels exactly in the
    CWM^T @ [P|1] decomposition)
  - msg segment-sum as a contiguous binary tree of bf16 adds instead of one
    strided tensor_reduce
"""

import sys

sys.path.insert(0, "/opt/trn_rl_repo")

import numpy as np
import ml_dtypes

BF16 = ml_dtypes.bfloat16

N = 64
C = 3
H = 128
L = 4
B = 128
NCORES = 8
GPC = B // NCORES  # graphs per core
E = N * N  # dense edges per graph

_CACHE = {}


def _prep_consts(inputs):
    """Numpy-side packing of weights into DMA-friendly layouts (replicated per core)."""
    f32 = np.float32
    ew1 = np.asarray(inputs["edge_w1"], f32)  # [L, 2H+1, H]
    d = {}
    d["w1a"] = np.concatenate([ew1[l, :H] for l in range(L)], axis=1).astype(BF16)
    d["w1b"] = np.concatenate([ew1[l, H : 2 * H] for l in range(L)], axis=1).astype(BF16)
    wrow = ew1[:, 2 * H]  # [L, 128]
    d["wm2l"] = np.concatenate([(-2.0 * wrow[l])[None, :] for l in range(L)], axis=1).astype(BF16)  # [1, 512]
    d["wrep"] = np.concatenate([np.tile(wrow[l][None, :], (N, 1)) for l in range(L)], axis=1).astype(np.float32)  # [64, 512]
    d["w2"] = np.concatenate([np.asarray(inputs["edge_w2"], f32)[l] for l in range(L)], axis=1).astype(BF16)
    d["cw1"] = np.concatenate([np.asarray(inputs["coord_w1"], f32)[l] for l in range(L)], axis=1).astype(BF16)
    d["cw2l"] = np.concatenate([np.asarray(inputs["coord_w2"], f32)[l] for l in range(L)], axis=1).astype(BF16)  # [128, 4]
    nw1 = np.asarray(inputs["node_w1"], f32)
    d["nw1a"] = np.concatenate([nw1[l, :H] for l in range(L)], axis=1).astype(BF16)
    d["nw1b"] = np.concatenate([nw1[l, H:] for l in range(L)], axis=1).astype(BF16)
    d["nw2"] = np.concatenate([np.asarray(inputs["node_w2"], f32)[l] for l in range(L)], axis=1).astype(BF16)
    # biases: [128, 5*L]; column blocks: edge_b1, edge_b2, coord_b1, node_b1, node_b2
    bias_cols = []
    for nm in ("edge_b1", "edge_b2", "coord_b1", "node_b1", "node_b2"):
        arr = np.asarray(inputs[nm], f32)  # [L, 128]
        for l in range(L):
            bias_cols.append(arr[l][:, None])
    d["biases"] = np.concatenate(bias_cols, axis=1)  # [128, 20]
    d["nerep"] = np.tile(np.asarray(inputs["node_embed"], f32).T, (1, N))  # [128, 64]
    d["ident"] = np.eye(N, dtype=f32)
    os_val = float(np.asarray(inputs["output_scale"], f32)[0])
    msc = np.zeros((N, 2), f32)
    msc[:, 0] = 1.0
    msc[:, 1] = os_val
    d["msc"] = msc
    d["inv64"] = np.full((1, N), 1.0 / N, f32)
    # selection matrix S [128, E]: rows 0-63 pick src i, rows 64-127 pick dst j
    S = np.zeros((2 * N, E), f32)
    ii = np.repeat(np.arange(N), N)
    jj = np.tile(np.arange(N), N)
    S[ii, np.arange(E)] = 1.0
    S[N + jj, np.arange(E)] = 1.0
    d["Sb"] = S.astype(BF16)
    return d


def _build(n_graphs, num_devices):
    import concourse.bacc as bacc
    import concourse.tile as tile
    import concourse.mybir as mybir

    dt = mybir.dt
    f32 = dt.float32
    bf16 = dt.bfloat16
    Silu = mybir.ActivationFunctionType.Silu
    add = mybir.AluOpType.add
    sub = mybir.AluOpType.subtract
    mult = mybir.AluOpType.mult

    nc = bacc.Bacc("TRN2", num_devices=num_devices, enable_partition_id=False)

    dr = {}
    for name, shape, cdt in [
        ("xin", [n_graphs, N, C], f32),
        ("xtin", [n_graphs, C, N], f32),
        ("Sb", [2 * N, E], bf16),
        ("w1a", [H, L * H], bf16),
        ("w1b", [H, L * H], bf16),
        ("wm2l", [1, L * H], bf16),
        ("wrep", [N, L * H], f32),
        ("w2", [H, L * H], bf16),
        ("cw1", [H, L * H], bf16),
        ("cw2l", [H, L], bf16),
        ("nw1a", [H, L * H], bf16),
        ("nw1b", [H, L * H], bf16),
        ("nw2", [H, L * H], bf16),
        ("biases", [H, 5 * L], f32),
        ("nerep", [H, N], f32),
        ("ident", [N, N], f32),
        ("msc", [N, 2], f32),
        ("inv64", [1, N], f32),
    ]:
        dr[name] = nc.dram_tensor(name, shape, cdt, kind="ExternalInput").ap()
    y = nc.dram_tensor("y", [n_graphs, N, C], f32, kind="ExternalOutput").ap()

    from contextlib import ExitStack
    with nc.allow_low_precision(reason="bf16 matmul pipeline"), tile.TileContext(nc) as tc, ExitStack() as es:
        cp = es.enter_context(tc.tile_pool(name="const", bufs=1))
        sp = es.enter_context(tc.tile_pool(name="state", bufs=1))
        wp = es.enter_context(tc.tile_pool(name="work", bufs=2))
        pp = es.enter_context(tc.tile_pool(name="epsum", bufs=3, space="PSUM"))
        sm = es.enter_context(tc.tile_pool(name="smps", bufs=2, space="PSUM"))

        # ---- load constants ----
        ct = {}
        for name in ("Sb", "w1a", "w1b", "wm2l", "wrep", "w2", "cw1", "cw2l",
                     "nw1a", "nw1b", "nw2", "biases", "nerep", "ident",
                     "msc", "inv64"):
            t = cp.tile(list(dr[name].shape), dr[name].dtype, tag=f"c_{name}")
            nc.sync.dma_start(out=t[:], in_=dr[name])
            ct[name] = t

        def wsl(name, l):  # [128,128] weight slice of layer l
            return ct[name][:, l * H : (l + 1) * H]

        def bsl(bi, l):  # bias column [128,1]
            return ct["biases"][:, bi * L + l : bi * L + l + 1]

        # ---- per-graph state ----
        HTs, HTbs, Pxs, PTs = [], [], [], []
        for g in range(n_graphs):
            HT = sp.tile([H, N], f32, tag=f"HT{g}")
            nc.vector.tensor_copy(out=HT[:], in_=ct["nerep"][:])
            HTb = sp.tile([H, N], bf16, tag=f"HTb{g}")
            nc.vector.tensor_copy(out=HTb[:], in_=ct["nerep"][:])
            Px = sp.tile([N, 4], f32, tag=f"Px{g}")
            nc.sync.dma_start(out=Px[:, 0:3], in_=dr["xin"][g])
            nc.vector.memset(Px[:, 3:4], 1.0)
            PT = sp.tile([C, N], f32, tag=f"PT{g}")
            nc.sync.dma_start(out=PT[:], in_=dr["xtin"][g])
            HTs.append(HT)
            HTbs.append(HTb)
            Pxs.append(Px)
            PTs.append(PT)

        for g in range(n_graphs):
            HT, HTb, Px, PT = HTs[g], HTbs[g], Pxs[g], PTs[g]
            for l in range(L):
                # ---- Gram row: Gram = P P^T, flattened via SBUF->SBUF DMA ----
                gram_ps = sm.tile([N, N], f32, tag="sm")
                nc.tensor.matmul(out=gram_ps[:], lhsT=PT[:], rhs=PT[:], start=True, stop=True)
                GS = wp.tile([N, N], bf16, tag="GS")
                nc.vector.tensor_copy(out=GS[:], in_=gram_ps[:])
                g_row = wp.tile([1, E], bf16, tag="g_row")
                nc.sync.dma_start(out=g_row[:], in_=GS[:])

                # gd = |p|^2 per node
                sq = wp.tile([N, C], f32, tag="sq")
                nc.vector.tensor_tensor(out=sq[:], in0=Px[:, 0:3], in1=Px[:, 0:3], op=mult)
                gd = wp.tile([N, 1], f32, tag="gd")
                nc.vector.tensor_reduce(out=gd[:], in_=sq[:], axis=mybir.AxisListType.X, op=add)

                # ---- A'/B' = H @ W1a|W1b, packed into one [128,128] psum ----
                ab_ps = sm.tile([H, H], f32, tag="sm")
                nc.tensor.matmul(out=ab_ps[0:N, :], lhsT=HTb[:], rhs=wsl("w1a", l), start=True, stop=True)
                nc.tensor.matmul(out=ab_ps[N : 2 * N, :], lhsT=HTb[:], rhs=wsl("w1b", l), start=True, stop=True)
                # lS = [A'; B'] + gd * wrow  (|p|^2 term folded into both halves)
                lS = wp.tile([2 * N, H], bf16, tag="lS")
                nc.vector.tensor_scalar_mul(out=lS[0:N, :], in0=ct["wrep"][:, l * H : (l + 1) * H], scalar1=gd[:])
                nc.vector.tensor_copy(out=lS[N:, :], in_=lS[0:N, :])
                nc.vector.tensor_tensor(out=lS[0:N, :], in0=lS[0:N, :], in1=ab_ps[0:N, :], op=add)
                nc.vector.tensor_tensor(out=lS[N:, :], in0=lS[N:, :], in1=ab_ps[N:, :], op=add)

                # ---- edge MLP stage 1: t1 = silu(A''[i] + B''[j] - 2*wrow*Gram + b1) ----
                t1 = wp.tile([H, E], bf16, tag="t1")
                for t in range(4):
                    ps = pp.tile([H, 1024], f32, tag="eps")
                    for q in range(2):
                        c0 = t * 1024 + q * 512
                        qs = slice(q * 512, (q + 1) * 512)
                        nc.tensor.matmul(out=ps[:, qs], lhsT=lS[:], rhs=ct["Sb"][:, c0 : c0 + 512],
                                         start=True, stop=False)
                        nc.tensor.matmul(out=ps[:, qs], lhsT=ct["wm2l"][:, l * H : (l + 1) * H],
                                         rhs=g_row[:, c0 : c0 + 512], start=False, stop=True)
                    nc.scalar.activation(out=t1[:, t * 1024 : (t + 1) * 1024], in_=ps[:], func=Silu, bias=bsl(0, l))

                # ---- stage 2: m = silu(t1 @ W2 + b2) ----
                m = wp.tile([H, E], bf16, tag="m")
                for t in range(4):
                    ps = pp.tile([H, 1024], f32, tag="eps")
                    for q in range(2):
                        c0 = t * 1024 + q * 512
                        nc.tensor.matmul(out=ps[:, q * 512 : (q + 1) * 512], lhsT=wsl("w2", l),
                                         rhs=t1[:, c0 : c0 + 512], start=True, stop=True)
                    nc.scalar.activation(out=m[:, t * 1024 : (t + 1) * 1024], in_=ps[:], func=Silu, bias=bsl(1, l))

                # ---- stage 3: t2 = silu(m @ CW1 + cb1) ----
                t2 = wp.tile([H, E], bf16, tag="t2")
                for t in range(4):
                    ps = pp.tile([H, 1024], f32, tag="eps")
                    for q in range(2):
                        c0 = t * 1024 + q * 512
                        nc.tensor.matmul(out=ps[:, q * 512 : (q + 1) * 512], lhsT=wsl("cw1", l),
                                         rhs=m[:, c0 : c0 + 512], start=True, stop=True)
                    nc.scalar.activation(out=t2[:, t * 1024 : (t + 1) * 1024], in_=ps[:], func=Silu, bias=bsl(2, l))

                # ---- stage 4: cw row = t2 @ cw2 ----
                cwS = wp.tile([1, E], bf16, tag="cwS")
                for t in range(4):
                    ps = pp.tile([H, 1024], f32, tag="eps")
                    for q in range(2):
                        c0 = t * 1024 + q * 512
                        nc.tensor.matmul(out=ps[0:1, q * 512 : (q + 1) * 512], lhsT=ct["cw2l"][:, l : l + 1],
                                         rhs=t2[:, c0 : c0 + 512], start=True, stop=True)
                    nc.vector.tensor_copy(out=cwS[:, t * 1024 : (t + 1) * 1024], in_=ps[0:1, :])

                # ---- pos update: upd = CWM^T @ [P|1]  (diag cancels exactly) ----
                CWM = wp.tile([N, N], bf16, tag="CWM")
                nc.sync.dma_start(out=CWM[:], in_=cwS[:])
                Pxb = wp.tile([N, 4], bf16, tag="Pxb")
                nc.vector.tensor_copy(out=Pxb[:], in_=Px[:])
                upd_ps = sm.tile([N, 4], f32, tag="sm")
                nc.tensor.matmul(out=upd_ps[:], lhsT=CWM[:], rhs=Pxb[:], start=True, stop=True)
                upds = wp.tile([N, 4], f32, tag="upds")
                nc.vector.tensor_copy(out=upds[:], in_=upd_ps[:])
                tmp = wp.tile([N, C], f32, tag="tmp")
                nc.vector.tensor_scalar_mul(out=tmp[:], in0=Px[:, 0:3], scalar1=upds[:, 3:4])
                nc.vector.tensor_tensor(out=Px[:, 0:3], in0=Px[:, 0:3], in1=upds[:, 0:3], op=add)
                nc.vector.tensor_tensor(out=Px[:, 0:3], in0=Px[:, 0:3], in1=tmp[:], op=sub)
                # refresh P^T
                ptp = sm.tile([C, N], f32, tag="sm")
                nc.tensor.transpose(out=ptp[:], in_=Px[:, 0:3], identity=ct["ident"][:])
                nc.vector.tensor_copy(out=PT[:], in_=ptp[:])

                # ---- msg_j = sum_i m[:, (i,j)] : contiguous binary tree ----
                ms1 = wp.tile([H, E // 2], bf16, tag="ms1")
                nc.vector.tensor_tensor(out=ms1[:], in0=m[:, : E // 2], in1=m[:, E // 2 :], op=add)
                ms2 = wp.tile([H, E // 4], bf16, tag="ms2")
                nc.vector.tensor_tensor(out=ms2[:], in0=ms1[:, : E // 4], in1=ms1[:, E // 4 :], op=add)
                ms3 = wp.tile([H, E // 8], bf16, tag="ms3")
                nc.vector.tensor_tensor(out=ms3[:], in0=ms2[:, : E // 8], in1=ms2[:, E // 8 :], op=add)
                ms4 = wp.tile([H, E // 16], bf16, tag="ms4")
                nc.vector.tensor_tensor(out=ms4[:], in0=ms3[:, : E // 16], in1=ms3[:, E // 16 :], op=add)
                ms5 = wp.tile([H, E // 32], bf16, tag="ms5")
                nc.vector.tensor_tensor(out=ms5[:], in0=ms4[:, : E // 32], in1=ms4[:, E // 32 :], op=add)
                msg = wp.tile([H, N], bf16, tag="msg")
                nc.vector.tensor_tensor(out=msg[:], in0=ms5[:, :N], in1=ms5[:, N:], op=add)
                nc.vector.tensor_tensor(out=msg[:], in0=msg[:], in1=m[:, :: N + 1], op=sub)

                # ---- node MLP ----
                nps = sm.tile([H, N], f32, tag="sm")
                nc.tensor.matmul(out=nps[:], lhsT=wsl("nw1a", l), rhs=HTb[:], start=True, stop=False)
                nc.tensor.matmul(out=nps[:], lhsT=wsl("nw1b", l), rhs=msg[:], start=False, stop=True)
                u = wp.tile([H, N], bf16, tag="u")
                nc.scalar.activation(out=u[:], in_=nps[:], func=Silu, bias=bsl(3, l))
                nps2 = sm.tile([H, N], f32, tag="sm")
                nc.tensor.matmul(out=nps2[:], lhsT=wsl("nw2", l), rhs=u[:], start=True, stop=True)
                nc.vector.tensor_tensor(out=HT[:], in0=HT[:], in1=nps2[:], op=add)
                nc.vector.tensor_scalar_add(out=HT[:], in0=HT[:], scalar1=bsl(4, l))
                nc.vector.tensor_copy(out=HTb[:], in_=HT[:])

            # ---- finalize graph g: dx = P - P0, mean-center, scale ----
            p0 = wp.tile([N, C], f32, tag="p0")
            nc.sync.dma_start(out=p0[:], in_=dr["xin"][g])
            dxt = wp.tile([N, C], f32, tag="dxt")
            nc.vector.tensor_tensor(out=dxt[:], in0=Px[:, 0:3], in1=p0[:], op=sub)
            mean = sm.tile([1, C], f32, tag="sm")
            nc.tensor.matmul(out=mean[:], lhsT=ct["msc"][:, 0:1], rhs=dxt[:], start=True, stop=True)
            means = wp.tile([1, C], f32, tag="means")
            nc.vector.tensor_copy(out=means[:], in_=mean[:])
            mrep = sm.tile([N, C], f32, tag="sm")
            nc.tensor.matmul(out=mrep[:], lhsT=ct["inv64"][:], rhs=means[:], start=True, stop=True)
            nc.vector.tensor_tensor(out=dxt[:], in0=dxt[:], in1=mrep[:], op=sub)
            nc.vector.tensor_scalar_mul(out=dxt[:], in0=dxt[:], scalar1=ct["msc"][:, 1:2])
            nc.sync.dma_start(out=y[g], in_=dxt[:])

    nc.compile()
    return nc


def _get_nc(n_graphs, num_devices):
    key = (n_graphs, num_devices)
    if key not in _CACHE:
        _CACHE[key] = _build(n_graphs, num_devices)
    return _CACHE[key]


def make_in_maps(inputs, n_graphs=GPC, ncores=NCORES):
    consts = _prep_consts(inputs)
    x = np.asarray(inputs["x"], np.float32)
    in_maps = []
    for c in range(ncores):
        xs = x[c * n_graphs : (c + 1) * n_graphs].reshape(n_graphs, N, C)
        m = dict(consts)
        m["xin"] = np.ascontiguousarray(xs)
        m["xtin"] = np.ascontiguousarray(xs.transpose(0, 2, 1))
        in_maps.append(m)
    return in_maps


def kernel(**inputs) -> np.ndarray:
    from concourse.bass_utils import run_bass_kernel_spmd

    nc = _get_nc(GPC, NCORES)
    in_maps = make_in_maps(inputs)
    res = run_bass_kernel_spmd(nc, in_maps, core_ids=list(range(NCORES)), trace=False)
    outs = [res.results[c]["y"].reshape(GPC, N * C) for c in range(NCORES)]
    return np.concatenate(outs, axis=0).astype(np.float32)


# revision 12
# speedup vs baseline: 536.4523x; 536.4523x over previous
"""Trainium2 Bass kernel for nn_CorrectorEGNN (B=128 graphs, N=64 nodes, H=128, L=4).

Strategy: data-parallel over graphs (16 graphs/core x 8 cores). Per graph the
fully-connected edge set is dense 64x64 ordered pairs (i-major, e = i*64+j,
src=i, dst=j). Edge tensors live channel-transposed: [128 chan (partitions),
4096 edges (free)], so the edge MLP is weight-stationary matmuls.

v1 (f16) changes vs the fp32r baseline:
  - all big matmuls in f16 (1 cyc/col vs fp32r's 2-pass + HAM-cold penalty)
  - |p|^2 folded into the edge-MLP first-layer lhsT (A' + gd*wrow), Gram term
    as a K=1 rank-1 matmul against the flattened Gram row
  - SBUF->SBUF DMA reshapes (Gram -> g_row, cw row -> CWM); no DRAM bounces
  - no diagonal zeroing of CWM (the i==j term cancels exactly in the
    CWM^T @ [P|1] decomposition)
  - msg segment-sum as a contiguous binary tree of f16 adds instead of one
    strided tensor_reduce
"""

import sys

sys.path.insert(0, "/opt/trn_rl_repo")

import numpy as np
import ml_dtypes

F16 = np.float16

N = 64
C = 3
H = 128
L = 4
B = 128
NCORES = 8
GPC = B // NCORES  # graphs per core
E = N * N  # dense edges per graph

_CACHE = {}


def _prep_consts(inputs):
    """Numpy-side packing of weights into DMA-friendly layouts (replicated per core)."""
    f32 = np.float32
    ew1 = np.asarray(inputs["edge_w1"], f32)  # [L, 2H+1, H]
    d = {}
    d["w1a"] = np.concatenate([ew1[l, :H] for l in range(L)], axis=1).astype(F16)
    d["w1b"] = np.concatenate([ew1[l, H : 2 * H] for l in range(L)], axis=1).astype(F16)
    wrow = ew1[:, 2 * H]  # [L, 128]
    d["wm2l"] = np.concatenate([(-2.0 * wrow[l])[None, :] for l in range(L)], axis=1).astype(F16)  # [1, 512]
    d["wrep"] = np.concatenate([np.tile(wrow[l][None, :], (N, 1)) for l in range(L)], axis=1).astype(np.float32)  # [64, 512]
    d["w2"] = np.concatenate([np.asarray(inputs["edge_w2"], f32)[l] for l in range(L)], axis=1).astype(F16)
    d["cw1"] = np.concatenate([np.asarray(inputs["coord_w1"], f32)[l] for l in range(L)], axis=1).astype(F16)
    d["cw2l"] = np.concatenate([np.asarray(inputs["coord_w2"], f32)[l] for l in range(L)], axis=1).astype(F16)  # [128, 4]
    nw1 = np.asarray(inputs["node_w1"], f32)
    d["nw1a"] = np.concatenate([nw1[l, :H] for l in range(L)], axis=1).astype(F16)
    d["nw1b"] = np.concatenate([nw1[l, H:] for l in range(L)], axis=1).astype(F16)
    d["nw2"] = np.concatenate([np.asarray(inputs["node_w2"], f32)[l] for l in range(L)], axis=1).astype(F16)
    # biases: [128, 5*L]; column blocks: edge_b1, edge_b2, coord_b1, node_b1, node_b2
    bias_cols = []
    for nm in ("edge_b1", "edge_b2", "coord_b1", "node_b1", "node_b2"):
        arr = np.asarray(inputs[nm], f32)  # [L, 128]
        for l in range(L):
            bias_cols.append(arr[l][:, None])
    d["biases"] = np.concatenate(bias_cols, axis=1)  # [128, 20]
    d["nerep"] = np.tile(np.asarray(inputs["node_embed"], f32).T, (1, N))  # [128, 64]
    d["ident"] = np.eye(N, dtype=f32)
    os_val = float(np.asarray(inputs["output_scale"], f32)[0])
    msc = np.zeros((N, 2), f32)
    msc[:, 0] = 1.0
    msc[:, 1] = os_val
    d["msc"] = msc
    d["inv64"] = np.full((1, N), 1.0 / N, f32)
    # selection matrix S [128, E]: rows 0-63 pick src i, rows 64-127 pick dst j
    S = np.zeros((2 * N, E), f32)
    ii = np.repeat(np.arange(N), N)
    jj = np.tile(np.arange(N), N)
    S[ii, np.arange(E)] = 1.0
    S[N + jj, np.arange(E)] = 1.0
    d["Sb"] = S.astype(F16)
    return d


def _build(n_graphs, num_devices):
    import concourse.bacc as bacc
    import concourse.tile as tile
    import concourse.mybir as mybir

    dt = mybir.dt
    f32 = dt.float32
    f16 = dt.float16
    Silu = mybir.ActivationFunctionType.Silu
    add = mybir.AluOpType.add
    sub = mybir.AluOpType.subtract
    mult = mybir.AluOpType.mult

    nc = bacc.Bacc("TRN2", num_devices=num_devices, enable_partition_id=False)

    dr = {}
    for name, shape, cdt in [
        ("xin", [n_graphs, N, C], f32),
        ("xtin", [n_graphs, C, N], f32),
        ("Sb", [2 * N, E], f16),
        ("w1a", [H, L * H], f16),
        ("w1b", [H, L * H], f16),
        ("wm2l", [1, L * H], f16),
        ("wrep", [N, L * H], f32),
        ("w2", [H, L * H], f16),
        ("cw1", [H, L * H], f16),
        ("cw2l", [H, L], f16),
        ("nw1a", [H, L * H], f16),
        ("nw1b", [H, L * H], f16),
        ("nw2", [H, L * H], f16),
        ("biases", [H, 5 * L], f32),
        ("nerep", [H, N], f32),
        ("ident", [N, N], f32),
        ("msc", [N, 2], f32),
        ("inv64", [1, N], f32),
    ]:
        dr[name] = nc.dram_tensor(name, shape, cdt, kind="ExternalInput").ap()
    y = nc.dram_tensor("y", [n_graphs, N, C], f32, kind="ExternalOutput").ap()

    from contextlib import ExitStack
    with nc.allow_low_precision(reason="f16 matmul pipeline"), tile.TileContext(nc) as tc, ExitStack() as es:
        cp = es.enter_context(tc.tile_pool(name="const", bufs=1))
        sp = es.enter_context(tc.tile_pool(name="state", bufs=1))
        wp = es.enter_context(tc.tile_pool(name="work", bufs=2))
        pp = es.enter_context(tc.tile_pool(name="epsum", bufs=3, space="PSUM"))
        sm = es.enter_context(tc.tile_pool(name="smps", bufs=2, space="PSUM"))

        # ---- load constants ----
        ct = {}
        for name in ("Sb", "w1a", "w1b", "wm2l", "wrep", "w2", "cw1", "cw2l",
                     "nw1a", "nw1b", "nw2", "biases", "nerep", "ident",
                     "msc", "inv64"):
            t = cp.tile(list(dr[name].shape), dr[name].dtype, tag=f"c_{name}")
            nc.sync.dma_start(out=t[:], in_=dr[name])
            ct[name] = t

        def wsl(name, l):  # [128,128] weight slice of layer l
            return ct[name][:, l * H : (l + 1) * H]

        def bsl(bi, l):  # bias column [128,1]
            return ct["biases"][:, bi * L + l : bi * L + l + 1]

        # ---- per-graph state ----
        HTs, HTbs, Pxs, PTs = [], [], [], []
        for g in range(n_graphs):
            HT = sp.tile([H, N], f32, tag=f"HT{g}")
            nc.vector.tensor_copy(out=HT[:], in_=ct["nerep"][:])
            HTb = sp.tile([H, N], f16, tag=f"HTb{g}")
            nc.vector.tensor_copy(out=HTb[:], in_=ct["nerep"][:])
            Px = sp.tile([N, 4], f32, tag=f"Px{g}")
            nc.sync.dma_start(out=Px[:, 0:3], in_=dr["xin"][g])
            nc.vector.memset(Px[:, 3:4], 1.0)
            PT = sp.tile([C, N], f32, tag=f"PT{g}")
            nc.sync.dma_start(out=PT[:], in_=dr["xtin"][g])
            HTs.append(HT)
            HTbs.append(HTb)
            Pxs.append(Px)
            PTs.append(PT)

        for g in range(n_graphs):
            HT, HTb, Px, PT = HTs[g], HTbs[g], Pxs[g], PTs[g]
            for l in range(L):
                # ---- Gram row: Gram = P P^T, flattened via SBUF->SBUF DMA ----
                gram_ps = sm.tile([N, N], f32, tag="sm")
                nc.tensor.matmul(out=gram_ps[:], lhsT=PT[:], rhs=PT[:], start=True, stop=True)
                GS = wp.tile([N, N], f16, tag="GS")
                nc.vector.tensor_copy(out=GS[:], in_=gram_ps[:])
                g_row = wp.tile([1, E], f16, tag="g_row")
                nc.sync.dma_start(out=g_row[:], in_=GS[:])

                # gd = |p|^2 per node
                sq = wp.tile([N, C], f32, tag="sq")
                nc.vector.tensor_tensor(out=sq[:], in0=Px[:, 0:3], in1=Px[:, 0:3], op=mult)
                gd = wp.tile([N, 1], f32, tag="gd")
                nc.vector.tensor_reduce(out=gd[:], in_=sq[:], axis=mybir.AxisListType.X, op=add)

                # ---- A'/B' = H @ W1a|W1b, packed into one [128,128] psum ----
                ab_ps = sm.tile([H, H], f32, tag="sm")
                nc.tensor.matmul(out=ab_ps[0:N, :], lhsT=HTb[:], rhs=wsl("w1a", l), start=True, stop=True)
                nc.tensor.matmul(out=ab_ps[N : 2 * N, :], lhsT=HTb[:], rhs=wsl("w1b", l), start=True, stop=True)
                # lS = [A'; B'] + gd * wrow  (|p|^2 term folded into both halves)
                lS = wp.tile([2 * N, H], f16, tag="lS")
                nc.vector.tensor_scalar_mul(out=lS[0:N, :], in0=ct["wrep"][:, l * H : (l + 1) * H], scalar1=gd[:])
                nc.vector.tensor_copy(out=lS[N:, :], in_=lS[0:N, :])
                nc.vector.tensor_tensor(out=lS[0:N, :], in0=lS[0:N, :], in1=ab_ps[0:N, :], op=add)
                nc.vector.tensor_tensor(out=lS[N:, :], in0=lS[N:, :], in1=ab_ps[N:, :], op=add)

                # ---- edge MLP stage 1: t1 = silu(A''[i] + B''[j] - 2*wrow*Gram + b1) ----
                t1 = wp.tile([H, E], f16, tag="t1")
                for t in range(4):
                    ps = pp.tile([H, 1024], f32, tag="eps")
                    for q in range(2):
                        c0 = t * 1024 + q * 512
                        qs = slice(q * 512, (q + 1) * 512)
                        nc.tensor.matmul(out=ps[:, qs], lhsT=lS[:], rhs=ct["Sb"][:, c0 : c0 + 512],
                                         start=True, stop=False)
                        nc.tensor.matmul(out=ps[:, qs], lhsT=ct["wm2l"][:, l * H : (l + 1) * H],
                                         rhs=g_row[:, c0 : c0 + 512], start=False, stop=True)
                    nc.scalar.activation(out=t1[:, t * 1024 : (t + 1) * 1024], in_=ps[:], func=Silu, bias=bsl(0, l))

                # ---- stage 2: m = silu(t1 @ W2 + b2) ----
                m = wp.tile([H, E], f16, tag="m")
                for t in range(4):
                    ps = pp.tile([H, 1024], f32, tag="eps")
                    for q in range(2):
                        c0 = t * 1024 + q * 512
                        nc.tensor.matmul(out=ps[:, q * 512 : (q + 1) * 512], lhsT=wsl("w2", l),
                                         rhs=t1[:, c0 : c0 + 512], start=True, stop=True)
                    nc.scalar.activation(out=m[:, t * 1024 : (t + 1) * 1024], in_=ps[:], func=Silu, bias=bsl(1, l))

                # ---- stage 3: t2 = silu(m @ CW1 + cb1) ----
                t2 = wp.tile([H, E], f16, tag="t2")
                for t in range(4):
                    ps = pp.tile([H, 1024], f32, tag="eps")
                    for q in range(2):
                        c0 = t * 1024 + q * 512
                        nc.tensor.matmul(out=ps[:, q * 512 : (q + 1) * 512], lhsT=wsl("cw1", l),
                                         rhs=m[:, c0 : c0 + 512], start=True, stop=True)
                    nc.scalar.activation(out=t2[:, t * 1024 : (t + 1) * 1024], in_=ps[:], func=Silu, bias=bsl(2, l))

                # ---- stage 4: cw row = t2 @ cw2 ----
                cwS = wp.tile([1, E], f16, tag="cwS")
                for t in range(4):
                    ps = pp.tile([H, 1024], f32, tag="eps")
                    for q in range(2):
                        c0 = t * 1024 + q * 512
                        nc.tensor.matmul(out=ps[0:1, q * 512 : (q + 1) * 512], lhsT=ct["cw2l"][:, l : l + 1],
                                         rhs=t2[:, c0 : c0 + 512], start=True, stop=True)
                    nc.vector.tensor_copy(out=cwS[:, t * 1024 : (t + 1) * 1024], in_=ps[0:1, :])

                # ---- pos update: upd = CWM^T @ [P|1]  (diag cancels exactly) ----
                CWM = wp.tile([N, N], f16, tag="CWM")
                nc.sync.dma_start(out=CWM[:], in_=cwS[:])
                Pxb = wp.tile([N, 4], f16, tag="Pxb")
                nc.vector.tensor_copy(out=Pxb[:], in_=Px[:])
                upd_ps = sm.tile([N, 4], f32, tag="sm")
                nc.tensor.matmul(out=upd_ps[:], lhsT=CWM[:], rhs=Pxb[:], start=True, stop=True)
                upds = wp.tile([N, 4], f32, tag="upds")
                nc.vector.tensor_copy(out=upds[:], in_=upd_ps[:])
                tmp = wp.tile([N, C], f32, tag="tmp")
                nc.vector.tensor_scalar_mul(out=tmp[:], in0=Px[:, 0:3], scalar1=upds[:, 3:4])
                nc.vector.tensor_tensor(out=Px[:, 0:3], in0=Px[:, 0:3], in1=upds[:, 0:3], op=add)
                nc.vector.tensor_tensor(out=Px[:, 0:3], in0=Px[:, 0:3], in1=tmp[:], op=sub)
                # refresh P^T
                ptp = sm.tile([C, N], f32, tag="sm")
                nc.tensor.transpose(out=ptp[:], in_=Px[:, 0:3], identity=ct["ident"][:])
                nc.vector.tensor_copy(out=PT[:], in_=ptp[:])

                # ---- msg_j = sum_i m[:, (i,j)] : contiguous binary tree ----
                ms1 = wp.tile([H, E // 2], f16, tag="ms1")
                nc.vector.tensor_tensor(out=ms1[:], in0=m[:, : E // 2], in1=m[:, E // 2 :], op=add)
                ms2 = wp.tile([H, E // 4], f16, tag="ms2")
                nc.vector.tensor_tensor(out=ms2[:], in0=ms1[:, : E // 4], in1=ms1[:, E // 4 :], op=add)
                ms3 = wp.tile([H, E // 8], f16, tag="ms3")
                nc.vector.tensor_tensor(out=ms3[:], in0=ms2[:, : E // 8], in1=ms2[:, E // 8 :], op=add)
                ms4 = wp.tile([H, E // 16], f16, tag="ms4")
                nc.vector.tensor_tensor(out=ms4[:], in0=ms3[:, : E // 16], in1=ms3[:, E // 16 :], op=add)
                ms5 = wp.tile([H, E // 32], f16, tag="ms5")
                nc.vector.tensor_tensor(out=ms5[:], in0=ms4[:, : E // 32], in1=ms4[:, E // 32 :], op=add)
                msg = wp.tile([H, N], f16, tag="msg")
                nc.vector.tensor_tensor(out=msg[:], in0=ms5[:, :N], in1=ms5[:, N:], op=add)
                nc.vector.tensor_tensor(out=msg[:], in0=msg[:], in1=m[:, :: N + 1], op=sub)

                # ---- node MLP ----
                nps = sm.tile([H, N], f32, tag="sm")
                nc.tensor.matmul(out=nps[:], lhsT=wsl("nw1a", l), rhs=HTb[:], start=True, stop=False)
                nc.tensor.matmul(out=nps[:], lhsT=wsl("nw1b", l), rhs=msg[:], start=False, stop=True)
                u = wp.tile([H, N], f16, tag="u")
                nc.scalar.activation(out=u[:], in_=nps[:], func=Silu, bias=bsl(3, l))
                nps2 = sm.tile([H, N], f32, tag="sm")
                nc.tensor.matmul(out=nps2[:], lhsT=wsl("nw2", l), rhs=u[:], start=True, stop=True)
                nc.vector.tensor_tensor(out=HT[:], in0=HT[:], in1=nps2[:], op=add)
                nc.vector.tensor_scalar_add(out=HT[:], in0=HT[:], scalar1=bsl(4, l))
                nc.vector.tensor_copy(out=HTb[:], in_=HT[:])

            # ---- finalize graph g: dx = P - P0, mean-center, scale ----
            p0 = wp.tile([N, C], f32, tag="p0")
            nc.sync.dma_start(out=p0[:], in_=dr["xin"][g])
            dxt = wp.tile([N, C], f32, tag="dxt")
            nc.vector.tensor_tensor(out=dxt[:], in0=Px[:, 0:3], in1=p0[:], op=sub)
            mean = sm.tile([1, C], f32, tag="sm")
            nc.tensor.matmul(out=mean[:], lhsT=ct["msc"][:, 0:1], rhs=dxt[:], start=True, stop=True)
            means = wp.tile([1, C], f32, tag="means")
            nc.vector.tensor_copy(out=means[:], in_=mean[:])
            mrep = sm.tile([N, C], f32, tag="sm")
            nc.tensor.matmul(out=mrep[:], lhsT=ct["inv64"][:], rhs=means[:], start=True, stop=True)
            nc.vector.tensor_tensor(out=dxt[:], in0=dxt[:], in1=mrep[:], op=sub)
            nc.vector.tensor_scalar_mul(out=dxt[:], in0=dxt[:], scalar1=ct["msc"][:, 1:2])
            nc.sync.dma_start(out=y[g], in_=dxt[:])

    nc.compile()
    return nc


def _get_nc(n_graphs, num_devices):
    key = (n_graphs, num_devices)
    if key not in _CACHE:
        _CACHE[key] = _build(n_graphs, num_devices)
    return _CACHE[key]


def make_in_maps(inputs, n_graphs=GPC, ncores=NCORES):
    consts = _prep_consts(inputs)
    x = np.asarray(inputs["x"], np.float32)
    in_maps = []
    for c in range(ncores):
        xs = x[c * n_graphs : (c + 1) * n_graphs].reshape(n_graphs, N, C)
        m = dict(consts)
        m["xin"] = np.ascontiguousarray(xs)
        m["xtin"] = np.ascontiguousarray(xs.transpose(0, 2, 1))
        in_maps.append(m)
    return in_maps


def _make_runner(nc, n_cores):
    """Build a persistent jitted executor: jit once, constants stay device-resident."""
    import jax
    from jax.sharding import Mesh, PartitionSpec, NamedSharding
    from jax.experimental.shard_map import shard_map
    from concourse.bass2jax import _bass_exec_p, install_neuronx_cc_hook
    import concourse.mybir as mybir

    install_neuronx_cc_hook()
    in_names, out_names, out_avals = [], [], []
    for alloc in nc.m.functions[0].allocations:
        if not isinstance(alloc, mybir.MemoryLocationSet):
            continue
        name = alloc.memorylocations[0].name
        if alloc.kind == "ExternalInput":
            in_names.append(name)
        elif alloc.kind == "ExternalOutput":
            out_names.append(name)
            out_avals.append(
                jax.core.ShapedArray(tuple(alloc.tensor_shape), mybir.dt.np(alloc.dtype))
            )
    n_params = len(in_names)
    all_names = tuple(in_names) + tuple(out_names)

    def _body(*args):
        outs = _bass_exec_p.bind(
            *args,
            out_avals=tuple(out_avals),
            in_names=all_names,
            out_names=tuple(out_names),
            lowering_input_output_aliases=(),
            sim_require_finite=True,
            sim_require_nnan=True,
            nc=nc,
        )
        return tuple(outs)

    devices = jax.devices()[:n_cores]
    mesh = Mesh(np.asarray(devices), ("core",))
    n_outs = len(out_names)
    in_specs = (PartitionSpec("core"),) * (n_params + n_outs)
    out_specs = (PartitionSpec("core"),) * n_outs
    donate = tuple(range(n_params, n_params + n_outs))
    fn = jax.jit(
        shard_map(_body, mesh=mesh, in_specs=in_specs, out_specs=out_specs, check_rep=False),
        donate_argnums=donate,
        keep_unused=True,
    )
    sharding = NamedSharding(mesh, PartitionSpec("core"))
    return {
        "fn": fn,
        "in_names": in_names,
        "out_avals": out_avals,
        "sharding": sharding,
        "jax": jax,
    }


def kernel(**inputs) -> np.ndarray:
    nc = _get_nc(GPC, NCORES)
    st = _CACHE.setdefault("rt", {})
    if "fn" not in st:
        st.update(_make_runner(nc, NCORES))
    jax = st["jax"]

    # constants: re-upload only when the weight inputs actually change
    import hashlib
    h = hashlib.md5()
    for k in sorted(inputs):
        if k not in ("x",):
            h.update(np.ascontiguousarray(inputs[k]).tobytes())
    key = h.hexdigest()
    if st.get("const_key") != key:
        consts = _prep_consts(inputs)
        dev = {}
        for name, arr in consts.items():
            big = np.concatenate([arr] * NCORES, axis=0)
            dev[name] = jax.device_put(big, st["sharding"])
        st["consts"] = dev
        st["const_key"] = key

    x = np.asarray(inputs["x"], np.float32)
    xs = np.ascontiguousarray(x.reshape(B, N, C))          # concat of per-core [GPC,N,C]
    xt = np.ascontiguousarray(xs.transpose(0, 2, 1))       # concat of per-core [GPC,C,N]
    args = []
    for name in st["in_names"]:
        if name == "xin":
            args.append(xs)
        elif name == "xtin":
            args.append(xt)
        else:
            args.append(st["consts"][name])
    zeros = [
        np.zeros((NCORES * a.shape[0], *a.shape[1:]), a.dtype) for a in st["out_avals"]
    ]
    outs = st["fn"](*args, *zeros)
    y = np.asarray(outs[0])  # [B, N, C]
    return np.ascontiguousarray(y.reshape(B, N * C)).astype(np.float32)
